# revision 51
# baseline (speedup 1.0000x reference)
"""Tensor-parallel causal self-attention (GQA + RoPE) for 8 Trainium2 cores.

Sharding: heads across cores. Each core gets 4 query heads + 1 KV head
(wq cols c*256:(c+1)*256, wk/wv cols c*64:(c+1)*64, wo rows c*256:(c+1)*256).
Each core computes a full [S, H] partial output (fp16); the host sums the 8
partials in fp64.

Device-side layouts are all "transposed" (channels on partitions):
  qT [dim, seq], kT [dim, seq] -> scores^T tiles [j, i] -> exp -> PV matmul
  with lhsT = [v | ones] giving attn_out^T and softmax denominators in one
  accumulation; attn_out^T is exactly the lhsT needed by the o_projection.

Head-dim interleave: RoPE pairs (d, d+32) are placed on adjacent partitions
(2d, 2d+1) via a host-side permutation of wq/wk columns and cos/sin rows, so
rotate_half is a DVE stream_shuffle (mask i^1) instead of SBUF->SBUF DMA.
Scores are invariant to the shared q/k permutation; v stays unpermuted.

Causality: only the lower trapezoid of scores^T is computed; the triangular
boundary sub-tile is masked multiplicatively (0/1, on GpSimd) after exp.
All HBM inputs are host-pre-arranged so every DMA is contiguous per
partition; output is stored fp16, one [128, H] store per seq tile.
"""

import json
import sys

import numpy as np

for _p in ("/opt/trn_rl_repo",):
    if _p not in sys.path:
        sys.path.insert(0, _p)

import concourse.bass as bass
import concourse.tile as tile
from concourse import mybir
from concourse.bass_utils import run_bass_kernel_spmd

B, S, H = 1, 2048, 2048
NH, NKV, HD = 32, 8, 64
ROPE_BASE = 10000.0
NCORES = 8
HQ = NH // NCORES            # 4 q heads per core
QW = HQ * HD                 # 256 q channels per core
NB = 512                     # xT streaming block width (seq positions)
IB = 512                     # attention i-block width
F32 = mybir.dt.float32
F16 = mybir.dt.float16
MMDT = F16                   # dtype for all matmul operands
MMNP = np.float16
KT = H // 128                # 16 contraction k-tiles for projections
NBLK = S // NB               # 4 xT blocks
IBLK = S // IB               # 4 attention i-blocks
JTN = S // 128               # 16 key j-tiles
SHUF_MASK = [i ^ 1 for i in range(32)]  # adjacent-pair swap per quadrant


def _split_multi_waits(bir_bytes: bytes) -> bytes:
    """This container's walrus accepts only one sync-wait per instruction;
    move extra waits onto preceding same-engine NoOps."""
    bir = json.loads(bir_bytes)
    n = [0]
    for fn in bir.get("functions", []):
        for bb in fn.get("blocks", []):
            insts = bb.get("instructions")
            if not insts:
                continue
            out = []
            for inst in insts:
                si = inst.get("sync_info")
                waits = (si or {}).get("on_wait") or []
                if len(waits) > 1:
                    for w in waits[:-1]:
                        n[0] += 1
                        out.append({
                            "debug": inst.get("debug", 0),
                            "engine": inst["engine"],
                            "ins": [], "outs": [],
                            "name": f"{inst['name']}-sw{n[0]}",
                            "opcode": "NoOp",
                            "sync_info": {"on_wait": [w], "on_update": []},
                        })
                    si["on_wait"] = waits[-1:]
                out.append(inst)
            bb["instructions"] = out
    return json.dumps(bir).encode()


def build_nc():
    nc = bass.Bass()

    xTd = nc.dram_tensor("xT", [128, NBLK, KT, NB], MMDT, kind="ExternalInput")
    wqd = nc.dram_tensor("wq", [128, KT, QW], MMDT, kind="ExternalInput")
    wkvd = nc.dram_tensor("wkv", [128, KT, 128], MMDT, kind="ExternalInput")
    wod = nc.dram_tensor("wo", [128, 2, H], MMDT, kind="ExternalInput")
    cosd = nc.dram_tensor("cosT", [128, S], MMDT, kind="ExternalInput")
    sind = nc.dram_tensor("sinT", [128, S], MMDT, kind="ExternalInput")
    trid = nc.dram_tensor("tri01", [128, 2, 128], MMDT, kind="ExternalInput")
    identd = nc.dram_tensor("ident64", [64, 64], MMDT, kind="ExternalInput")
    onesd = nc.dram_tensor("ones", [128, 129], MMDT, kind="ExternalInput")
    out_d = nc.dram_tensor("out", [S, H], MMDT, kind="ExternalOutput")

    with tile.TileContext(nc) as tc:
        with (
            tc.tile_pool(name="const", bufs=1) as cpool,
            tc.tile_pool(name="xin", bufs=3) as xpool,
            tc.tile_pool(name="tmp", bufs=3) as tpool,
            tc.tile_pool(name="ex", bufs=4) as expool,
            tc.tile_pool(name="ao", bufs=4) as aopool,
            tc.tile_pool(name="ostage", bufs=3) as opool,
            tc.tile_pool(name="ps_pj", bufs=2, space="PSUM") as ps_pj,
            tc.tile_pool(name="ps_sc", bufs=2, space="PSUM") as ps_sc,
            tc.tile_pool(name="ps_pv", bufs=2, space="PSUM") as ps_pv,
        ):
            # ---- persistent SBUF tiles ----
            wq_sb = cpool.tile([128, KT, QW], MMDT)
            wkv_sb = cpool.tile([128, KT, 128], MMDT)
            cos_sb = cpool.tile([128, S], MMDT)
            sin_sb = cpool.tile([128, S], MMDT)
            tri_sb = cpool.tile([128, 2, 128], MMDT)
            ident = cpool.tile([64, 64], MMDT)
            ones128 = cpool.tile([1, 128], MMDT)
            qT_sb = cpool.tile([128, 2, S], MMDT)      # heads (0,1 | 2,3)
            kT_sb = cpool.tile([128, S], MMDT)         # kT duplicated on both halves
            vnat_sb = cpool.tile([128, JTN, HD + 1], MMDT)
            aoT_sb = cpool.tile([128, 2, S], MMDT)     # attn_out^T (o_proj lhsT)
            wo_sb = cpool.tile([128, 2, H], MMDT)

            def rope(dst, src, nb, plo, phi):
                """dst = src*cosS + shuffle(src)*sinS on partitions plo:phi
                (sign of rotate_half baked into sinS; pair dims adjacent)."""
                sl = bass.ts(nb, NB)
                rot = tpool.tile([128, NB], MMDT, tag="rot", name="rot")
                nc.vector.stream_shuffle(rot[plo:phi, :], src, SHUF_MASK)
                m1 = tpool.tile([128, NB], MMDT, tag="m1", name="m1")
                nc.vector.tensor_tensor(m1[plo:phi, :], src, cos_sb[plo:phi, sl],
                                        mybir.AluOpType.mult)
                m2 = tpool.tile([128, NB], MMDT, tag="m2", name="m2")
                nc.vector.tensor_tensor(m2[plo:phi, :], rot[plo:phi, :],
                                        sin_sb[plo:phi, sl], mybir.AluOpType.mult)
                nc.vector.tensor_tensor(dst, m1[plo:phi, :], m2[plo:phi, :],
                                        mybir.AluOpType.add)

            CH = 4  # proj matmuls per filler chunk

            def make_proj_chunks(nb):
                """Prefetch xT block now; return callables that emit the
                projection matmuls/evictions/rope piecewise."""
                sl = bass.ts(nb, NB)
                xt = xpool.tile([128, KT, NB], MMDT, tag="xt", name=f"xt{nb}")
                if nb == 0:  # interleave with wq pieces so the first matmuls
                    for kp in range(0, KT, 4):  # start as data trickles in
                        nc.sync.dma_start(wq_sb[:, kp:kp + 4, :],
                                          wqd[:, kp:kp + 4, :])
                        nc.sync.dma_start(xt[:, kp:kp + 4, :],
                                          xTd[:, nb, kp:kp + 4, :])
                else:
                    nc.sync.dma_start(xt[:, 0:8, :], xTd[:, nb, 0:8, :])
                    nc.sync.dma_start(xt[:, 8:KT, :], xTd[:, nb, 8:KT, :])
                kvt = tpool.tile([128, NB], MMDT, tag="kvt", name=f"kvt{nb}")
                chunks = []
                for mt in range(3):  # 0,1 = q m-tiles; 2 = kv (v|k)
                    pj = ps_pj.tile([128, NB], F32, tag="pj", name=f"pj_{nb}_{mt}")
                    w_sb = wkv_sb if mt == 2 else wq_sb

                    def mk_mm(k0, mt=mt, pj=pj, w_sb=w_sb):
                        def emit():
                            for k in range(k0, min(k0 + CH, KT)):
                                wsl = w_sb[:, k, :] if mt == 2 else \
                                    w_sb[:, k, bass.ts(mt, 128)]
                                nc.tensor.matmul(pj[:], wsl, xt[:, k, :],
                                                 start=(k == 0), stop=(k == KT - 1))
                        return emit
                    for k0 in range(0, KT, CH):
                        chunks.append(mk_mm(k0))

                    if mt < 2:
                        def ev(mt=mt, pj=pj):
                            qtmp = tpool.tile([128, NB], MMDT, tag="qtmp", name="qtmp")
                            nc.vector.tensor_copy(qtmp[:], pj[:])
                            rope(qT_sb[:, mt, sl], qtmp[:], nb, 0, 128)
                        chunks.append(ev)
                    else:
                        def evkv(pj=pj):
                            # pj: v on partitions 0:64, k on 64:128
                            nc.vector.tensor_copy(kvt[:], pj[:])
                            rope(kT_sb[64:128, sl], kvt[64:128, :], nb, 64, 128)
                            nc.sync.dma_start(kT_sb[0:64, sl], kT_sb[64:128, sl])
                        chunks.append(evkv)

                def tpc_mk(jj):
                    def tpc():
                        jt = (nb * NB) // 128 + jj
                        # pj-ring slot: its consumers (DVE evictions) drain
                        # fast, unlike the exp-gated sc ring.
                        tp = ps_pj.tile([128, HD], MMDT, tag="pj", name="tp")
                        nc.tensor.transpose(tp[:], kvt[0:64, bass.ts(jj, 128)],
                                            ident[:])
                        nc.vector.tensor_copy(vnat_sb[:, jt, 0:HD], tp[:])
                    return tpc
                for jj in range(NB // 128):
                    chunks.append(tpc_mk(jj))
                return chunks

            def make_oproj_chunks(it):
                last = it == IBLK - 1   # tail: nothing left to overlap, so
                chunks = []             # split evictions DVE/ACT + both queues
                for sti in range(it * (IB // 128), (it + 1) * (IB // 128)):
                    og = opool.tile([128, H], MMDT, tag="og", name=f"og{sti}")
                    for eb in range(H // 512):
                        def opc(sti=sti, eb=eb, og=og):
                            ssl = bass.ts(sti, 128)
                            op = ps_pj.tile([128, 512], F32, tag="pj", name="op")
                            nc.tensor.matmul(op[:], aoT_sb[:, 0, ssl],
                                             wo_sb[:, 0, bass.ts(eb, 512)],
                                             start=True, stop=False)
                            nc.tensor.matmul(op[:], aoT_sb[:, 1, ssl],
                                             wo_sb[:, 1, bass.ts(eb, 512)],
                                             start=False, stop=True)
                            if last and eb % 2 == 1:
                                nc.scalar.copy(og[:, bass.ts(eb, 512)], op[:])
                            else:
                                nc.vector.tensor_copy(og[:, bass.ts(eb, 512)], op[:])
                            if eb == H // 512 - 1:
                                if last and sti % 2 == 1:
                                    nc.scalar.dma_start(out_d[ssl, :], og[:])
                                else:
                                    nc.sync.dma_start(out_d[ssl, :], og[:])
                        chunks.append(opc)
                return chunks

            def attention(it, chunks):
                i_lo = it * IB
                njt = (it + 1) * (IB // 128)
                norm_chunks = []

                def emit_scores(mt, jt):
                    i0 = max(0, jt * 128 - i_lo)
                    st = ps_sc.tile([128, 2, IB], F32, tag="sc", name="st")
                    nc.tensor.matmul(
                        st[:, 0, i0:IB], kT_sb[0:64, bass.ts(jt, 128)],
                        qT_sb[0:64, mt, i_lo + i0:i_lo + IB],
                        start=True, stop=True)
                    nc.tensor.matmul(
                        st[:, 1, i0:IB], kT_sb[64:128, bass.ts(jt, 128)],
                        qT_sb[64:128, mt, i_lo + i0:i_lo + IB],
                        start=True, stop=True)
                    ex = expool.tile([128, 2, IB], MMDT, tag="ex", name="ex")
                    nc.scalar.activation(
                        ex[:, :, i0:IB], st[:, :, i0:IB],
                        mybir.ActivationFunctionType.Exp, scale=1.0 / 8.0)
                    if jt * 128 >= i_lo:  # mask the triangular boundary
                        nc.gpsimd.tensor_tensor(
                            ex[:, :, i0:i0 + 128], ex[:, :, i0:i0 + 128],
                            tri_sb[:], mybir.AluOpType.mult)
                    return ex, i0

                for mt in range(2):  # head pair on partitions 0:64 / 64:128
                    pv0 = ps_pv.tile([HD + 1, IB], F32, tag="pv", name="pv0")
                    pv1 = ps_pv.tile([HD + 1, IB], F32, tag="pv", name="pv1")
                    # software-pipelined by one j-tile: scores(jt+1) is emitted
                    # before pv(jt) so exp(jt) latency hides behind real PE work
                    pipe = emit_scores(mt, 0)
                    for jt in range(njt):
                        if jt + 1 < njt:
                            nxt = emit_scores(mt, jt + 1)
                        else:
                            nxt = None
                        if chunks:
                            chunks.pop(0)()  # PE filler while ACT runs exp
                        ex, i0 = pipe
                        nc.tensor.matmul(
                            pv0[:, i0:IB], vnat_sb[:, jt, :], ex[:, 0, i0:IB],
                            start=(jt == 0), stop=(jt == njt - 1))
                        nc.tensor.matmul(
                            pv1[:, i0:IB], vnat_sb[:, jt, :], ex[:, 1, i0:IB],
                            start=(jt == 0), stop=(jt == njt - 1))
                        pipe = nxt
                    # softmax denominators: ln from PSUM, reciprocal via exp(-x)
                    lz = aopool.tile([1, 2 * IB], F32, tag="lz", name="lz")
                    nc.scalar.activation(lz[0:1, 0:IB], pv0[HD:HD + 1, :],
                                         mybir.ActivationFunctionType.Ln)
                    nc.scalar.activation(lz[0:1, IB:2 * IB], pv1[HD:HD + 1, :],
                                         mybir.ActivationFunctionType.Ln)
                    rr = aopool.tile([1, 2 * IB], MMDT, tag="rr", name="rr")
                    nc.scalar.activation(rr[:], lz[:],
                                         mybir.ActivationFunctionType.Exp, scale=-1.0)
                    ao = aopool.tile([128, IB], F32, tag="ao", name="ao")
                    nc.vector.tensor_copy(ao[0:HD, :], pv0[0:HD, :])
                    nc.scalar.copy(ao[64:64 + HD, :], pv1[0:HD, :])

                    # defer the bc matmuls + normalize so they don't
                    # head-of-line-block the PE queue behind the ACT chain
                    def norm(mt=mt, rr=rr, ao=ao):
                        # sc-ring slots: st accumulations are single-emission,
                        # so a popped-in bc write can never split an open chain
                        bc0 = ps_sc.tile([128, 2, IB], F32, tag="sc", name="bc0")
                        nc.tensor.matmul(bc0[:, 0, :], ones128[:], rr[0:1, 0:IB],
                                         start=True, stop=True)
                        nc.vector.tensor_tensor(
                            aoT_sb[0:HD, mt, i_lo:i_lo + IB], ao[0:HD, :],
                            bc0[0:HD, 0, :], mybir.AluOpType.mult)
                        bc1 = ps_sc.tile([128, 2, IB], F32, tag="sc", name="bc1")
                        nc.tensor.matmul(bc1[:, 0, :], ones128[:],
                                         rr[0:1, IB:2 * IB],
                                         start=True, stop=True)
                        nc.vector.tensor_tensor(
                            aoT_sb[64:64 + HD, mt, i_lo:i_lo + IB],
                            ao[64:64 + HD, :], bc1[64:128, 0, :],
                            mybir.AluOpType.mult)
                    if mt == 0:
                        chunks.insert(min(2, len(chunks)), norm)
                    else:
                        norm_chunks.append(norm)
                return norm_chunks

            # ---- main pipeline ----
            # startup: sync queue streams wq/xt0 interleaved then wkv
            # (compute-critical path); scalar queue streams rope tables +
            # constants + wo in parallel.
            chunks0 = make_proj_chunks(0)      # issues wq+xt0 DMAs
            nc.sync.dma_start(wkv_sb[:], wkvd[:])
            nc.scalar.dma_start(cos_sb[:], cosd[:])
            nc.scalar.dma_start(sin_sb[:], sind[:])
            nc.scalar.dma_start(ident[:], identd[:])
            nc.scalar.dma_start(ones128[:], onesd[0:1, 0:128])
            nc.vector.memset(vnat_sb[:, :, HD:HD + 1], 1.0)
            nc.scalar.dma_start(tri_sb[:], trid[:])
            nc.scalar.dma_start(wo_sb[:], wod[:])
            for c in chunks0:
                c()
            opq = []                     # deferred o_proj chunks
            norm_pending = []
            for it in range(IBLK):
                chunks = list(norm_pending)
                take = (0, 8, 16, 24)[it]
                chunks += opq[:take]
                opq = opq[take:]
                if it + 1 < NBLK:
                    chunks += make_proj_chunks(it + 1)
                norm_pending = attention(it, chunks)
                for c in chunks:  # drain leftover proj work before next i-block
                    c()
                opq += make_oproj_chunks(it)
            for c in norm_pending:
                c()
            for c in opq:
                c()

    orig = nc.to_json_bytes
    nc.to_json_bytes = lambda: _split_multi_waits(orig())
    return nc


PERM64 = np.empty(64, dtype=np.int64)
PERM64[0::2] = np.arange(32)          # partition 2j   <- dim j
PERM64[1::2] = np.arange(32) + 32     # partition 2j+1 <- dim j+32


def _host_tables(position_ids):
    pos = np.asarray(position_ids).reshape(-1).astype(np.float64)
    inv = 1.0 / (ROPE_BASE ** (np.arange(0, HD, 2, dtype=np.float64) / HD))  # [32]
    fr = pos[None, :] * inv[:, None]                        # [32, S]
    c64 = np.empty((64, len(pos)))
    c64[0::2] = np.cos(fr)
    c64[1::2] = np.cos(fr)
    s64 = np.empty((64, len(pos)))
    s64[0::2] = -np.sin(fr)                                 # sign of rotate_half
    s64[1::2] = np.sin(fr)
    cosT = np.vstack([c64, c64]).astype(MMNP)               # [128, S]
    sinT = np.vstack([s64, s64]).astype(MMNP)
    tri = (np.arange(128)[:, None] <= np.arange(128)[None, :])
    tri01 = np.broadcast_to(tri[:, None, :], (128, 2, 128)).astype(MMNP)
    return cosT, sinT, np.ascontiguousarray(tri01)


def _perm_heads(w):
    """Permute each 64-col head block of w by PERM64 (pair dims adjacent)."""
    out = w.reshape(w.shape[0], -1, 64)[:, :, PERM64]
    return out.reshape(w.shape[0], -1)


_NC_CACHE = {}


def kernel(**inputs):
    x = np.asarray(inputs["x"], dtype=np.float32)
    wq = np.asarray(inputs["wq"], dtype=np.float32)
    wk = np.asarray(inputs["wk"], dtype=np.float32)
    wv = np.asarray(inputs["wv"], dtype=np.float32)
    wo = np.asarray(inputs["wo"], dtype=np.float32)
    cosT, sinT, tri01 = _host_tables(inputs["position_ids"])
    # x [S,H] -> [p, nb, ko, s'] so each per-partition DMA row is contiguous
    xTd = np.ascontiguousarray(
        x.reshape(NBLK, NB, KT, 128).transpose(3, 0, 2, 1)).astype(MMNP)

    if "nc" not in _NC_CACHE:
        _NC_CACHE["nc"] = build_nc()
    nc = _NC_CACHE["nc"]

    wqp = _perm_heads(wq)
    wkp = _perm_heads(wk)
    in_maps = []
    for c in range(NCORES):
        wq_c = wqp[:, c * QW:(c + 1) * QW]
        wkv_c = np.concatenate([wv[:, c * HD:(c + 1) * HD],
                                wkp[:, c * HD:(c + 1) * HD]], axis=1)  # [v | k]
        wo_c = wo[c * QW:(c + 1) * QW, :]
        in_maps.append({
            "xT": xTd,
            "wq": np.ascontiguousarray(
                wq_c.reshape(KT, 128, QW).transpose(1, 0, 2)).astype(MMNP),
            "wkv": np.ascontiguousarray(
                wkv_c.reshape(KT, 128, 128).transpose(1, 0, 2)).astype(MMNP),
            "wo": np.ascontiguousarray(
                wo_c.reshape(2, 128, H).transpose(1, 0, 2)).astype(MMNP),
            "cosT": cosT, "sinT": sinT, "tri01": tri01,
            "ident64": np.eye(64, dtype=MMNP),
            "ones": np.ones((128, 129), dtype=MMNP),
        })
    res = run_bass_kernel_spmd(nc, in_maps, core_ids=list(range(NCORES)))
    acc = np.zeros((S, H), dtype=np.float64)
    for c in range(NCORES):
        acc += res.results[c]["out"].astype(np.float64)
    return acc.astype(np.float32).reshape(B, S, H)


if __name__ == "__main__":
    rng = np.random.default_rng(0)
    ins = {
        "x": rng.standard_normal((B, S, H), dtype=np.float32),
        "position_ids": np.broadcast_to(np.arange(S, dtype=np.int64), (B, S)),
        "wq": (rng.standard_normal((H, NH * HD), dtype=np.float32) * 0.02),
        "wk": (rng.standard_normal((H, NKV * HD), dtype=np.float32) * 0.02),
        "wv": (rng.standard_normal((H, NKV * HD), dtype=np.float32) * 0.02),
        "wo": (rng.standard_normal((NH * HD, H), dtype=np.float32) * 0.02),
    }
    out = kernel(**ins)
    print(out.shape, out.dtype, np.abs(out).mean())


# revision 52
# speedup vs baseline: 1.0172x; 1.0172x over previous
"""Tensor-parallel causal self-attention (GQA + RoPE) for 8 Trainium2 cores.

Sharding: heads across cores. Each core gets 4 query heads + 1 KV head
(wq cols c*256:(c+1)*256, wk/wv cols c*64:(c+1)*64, wo rows c*256:(c+1)*256).
Each core computes a full [S, H] partial output (fp16); the host sums the 8
partials in fp64.

Device-side layouts are all "transposed" (channels on partitions):
  qT [dim, seq], kT [dim, seq] -> scores^T tiles [j, i] -> exp -> PV matmul
  with lhsT = [v | ones] giving attn_out^T and softmax denominators in one
  accumulation; attn_out^T is exactly the lhsT needed by the o_projection.

Head-dim interleave: RoPE pairs (d, d+32) are placed on adjacent partitions
(2d, 2d+1) via a host-side permutation of wq/wk columns and cos/sin rows, so
rotate_half is a DVE stream_shuffle (mask i^1) instead of SBUF->SBUF DMA.
Scores are invariant to the shared q/k permutation; v stays unpermuted.

Causality: only the lower trapezoid of scores^T is computed; the triangular
boundary sub-tile is masked multiplicatively (0/1, on GpSimd) after exp.
All HBM inputs are host-pre-arranged so every DMA is contiguous per
partition; output is stored fp16, one [128, H] store per seq tile.
"""

import json
import sys

import numpy as np

for _p in ("/opt/trn_rl_repo",):
    if _p not in sys.path:
        sys.path.insert(0, _p)

import concourse.bass as bass
import concourse.tile as tile
from concourse import mybir
from concourse.bass_utils import run_bass_kernel_spmd

B, S, H = 1, 2048, 2048
NH, NKV, HD = 32, 8, 64
ROPE_BASE = 10000.0
NCORES = 8
HQ = NH // NCORES            # 4 q heads per core
QW = HQ * HD                 # 256 q channels per core
NB = 512                     # xT streaming block width (seq positions)
IB = 512                     # attention i-block width
F32 = mybir.dt.float32
F16 = mybir.dt.float16
MMDT = F16                   # dtype for all matmul operands
MMNP = np.float16
KT = H // 128                # 16 contraction k-tiles for projections
NBLK = S // NB               # 4 xT blocks
IBLK = S // IB               # 4 attention i-blocks
JTN = S // 128               # 16 key j-tiles
SHUF_MASK = [i ^ 1 for i in range(32)]  # adjacent-pair swap per quadrant


def _split_multi_waits(bir_bytes: bytes) -> bytes:
    """This container's walrus accepts only one sync-wait per instruction;
    move extra waits onto preceding same-engine NoOps."""
    bir = json.loads(bir_bytes)
    n = [0]
    for fn in bir.get("functions", []):
        for bb in fn.get("blocks", []):
            insts = bb.get("instructions")
            if not insts:
                continue
            out = []
            for inst in insts:
                si = inst.get("sync_info")
                waits = (si or {}).get("on_wait") or []
                if len(waits) > 1:
                    for w in waits[:-1]:
                        n[0] += 1
                        out.append({
                            "debug": inst.get("debug", 0),
                            "engine": inst["engine"],
                            "ins": [], "outs": [],
                            "name": f"{inst['name']}-sw{n[0]}",
                            "opcode": "NoOp",
                            "sync_info": {"on_wait": [w], "on_update": []},
                        })
                    si["on_wait"] = waits[-1:]
                out.append(inst)
            bb["instructions"] = out
    return json.dumps(bir).encode()


def build_nc():
    nc = bass.Bass()

    xTd = nc.dram_tensor("xT", [128, NBLK, KT, NB], MMDT, kind="ExternalInput")
    wqd = nc.dram_tensor("wq", [128, KT, QW], MMDT, kind="ExternalInput")
    wkvd = nc.dram_tensor("wkv", [128, KT, 128], MMDT, kind="ExternalInput")
    wod = nc.dram_tensor("wo", [128, 2, H], MMDT, kind="ExternalInput")
    cosd = nc.dram_tensor("cosT", [128, S], MMDT, kind="ExternalInput")
    sind = nc.dram_tensor("sinT", [128, S], MMDT, kind="ExternalInput")
    trid = nc.dram_tensor("tri01", [128, 2, 128], MMDT, kind="ExternalInput")
    identd = nc.dram_tensor("ident64", [64, 64], MMDT, kind="ExternalInput")
    onesd = nc.dram_tensor("ones", [128, 129], MMDT, kind="ExternalInput")
    out_d = nc.dram_tensor("out", [S, H], MMDT, kind="ExternalOutput")

    with tile.TileContext(nc) as tc:
        with (
            tc.tile_pool(name="const", bufs=1) as cpool,
            tc.tile_pool(name="xin", bufs=3) as xpool,
            tc.tile_pool(name="tmp", bufs=3) as tpool,
            tc.tile_pool(name="ex", bufs=4) as expool,
            tc.tile_pool(name="ao", bufs=4) as aopool,
            tc.tile_pool(name="ostage", bufs=3) as opool,
            tc.tile_pool(name="ps_pj", bufs=2, space="PSUM") as ps_pj,
            tc.tile_pool(name="ps_sc", bufs=2, space="PSUM") as ps_sc,
            tc.tile_pool(name="ps_pv", bufs=2, space="PSUM") as ps_pv,
        ):
            # ---- persistent SBUF tiles ----
            wq_sb = cpool.tile([128, KT, QW], MMDT)
            wkv_sb = cpool.tile([128, KT, 128], MMDT)
            cos_sb = cpool.tile([128, S], MMDT)
            sin_sb = cpool.tile([128, S], MMDT)
            tri_sb = cpool.tile([128, 2, 128], MMDT)
            ident = cpool.tile([64, 64], MMDT)
            ones128 = cpool.tile([1, 128], MMDT)
            qT_sb = cpool.tile([128, 2, S], MMDT)      # heads (0,1 | 2,3)
            kT_sb = cpool.tile([128, S], MMDT)         # kT duplicated on both halves
            vnat_sb = cpool.tile([128, JTN, HD + 1], MMDT)
            aoT_sb = cpool.tile([128, 2, S], MMDT)     # attn_out^T (o_proj lhsT)
            wo_sb = cpool.tile([128, 2, H], MMDT)

            def rope(dst, src, nb, plo, phi):
                """dst = src*cosS + shuffle(src)*sinS on partitions plo:phi
                (sign of rotate_half baked into sinS; pair dims adjacent)."""
                sl = bass.ts(nb, NB)
                rot = tpool.tile([128, NB], MMDT, tag="rot", name="rot")
                nc.vector.stream_shuffle(rot[plo:phi, :], src, SHUF_MASK)
                m1 = tpool.tile([128, NB], MMDT, tag="m1", name="m1")
                nc.vector.tensor_tensor(m1[plo:phi, :], src, cos_sb[plo:phi, sl],
                                        mybir.AluOpType.mult)
                m2 = tpool.tile([128, NB], MMDT, tag="m2", name="m2")
                nc.vector.tensor_tensor(m2[plo:phi, :], rot[plo:phi, :],
                                        sin_sb[plo:phi, sl], mybir.AluOpType.mult)
                nc.vector.tensor_tensor(dst, m1[plo:phi, :], m2[plo:phi, :],
                                        mybir.AluOpType.add)

            CH = 4  # proj matmuls per filler chunk

            def make_proj_chunks(nb):
                """Prefetch xT block now; return callables that emit the
                projection matmuls/evictions/rope piecewise."""
                sl = bass.ts(nb, NB)
                xt = xpool.tile([128, KT, NB], MMDT, tag="xt", name=f"xt{nb}")
                if nb == 0:  # interleave with wq pieces so the first matmuls
                    for kp in range(0, KT, 4):  # start as data trickles in
                        nc.sync.dma_start(wq_sb[:, kp:kp + 4, :],
                                          wqd[:, kp:kp + 4, :])
                        nc.sync.dma_start(xt[:, kp:kp + 4, :],
                                          xTd[:, nb, kp:kp + 4, :])
                else:
                    nc.sync.dma_start(xt[:, 0:8, :], xTd[:, nb, 0:8, :])
                    nc.sync.dma_start(xt[:, 8:KT, :], xTd[:, nb, 8:KT, :])
                kvt = tpool.tile([128, NB], MMDT, tag="kvt", name=f"kvt{nb}")
                chunks = []
                for mt in range(3):  # 0,1 = q m-tiles; 2 = kv (v|k)
                    pj = ps_pj.tile([128, NB], F32, tag="pj", name=f"pj_{nb}_{mt}")
                    w_sb = wkv_sb if mt == 2 else wq_sb

                    def mk_mm(k0, mt=mt, pj=pj, w_sb=w_sb):
                        def emit():
                            for k in range(k0, min(k0 + CH, KT)):
                                wsl = w_sb[:, k, :] if mt == 2 else \
                                    w_sb[:, k, bass.ts(mt, 128)]
                                nc.tensor.matmul(pj[:], wsl, xt[:, k, :],
                                                 start=(k == 0), stop=(k == KT - 1))
                        return emit
                    for k0 in range(0, KT, CH):
                        chunks.append(mk_mm(k0))

                    if mt < 2:
                        def ev(mt=mt, pj=pj):
                            qtmp = tpool.tile([128, NB], MMDT, tag="qtmp", name="qtmp")
                            nc.vector.tensor_copy(qtmp[:], pj[:])
                            rope(qT_sb[:, mt, sl], qtmp[:], nb, 0, 128)
                        chunks.append(ev)
                    else:
                        def evkv(pj=pj):
                            # pj: v on partitions 0:64, k on 64:128
                            nc.vector.tensor_copy(kvt[:], pj[:])
                            rope(kT_sb[64:128, sl], kvt[64:128, :], nb, 64, 128)
                            nc.sync.dma_start(kT_sb[0:64, sl], kT_sb[64:128, sl])
                        chunks.append(evkv)

                def tpc_mk(jj):
                    def tpc():
                        jt = (nb * NB) // 128 + jj
                        # pj-ring slot: its consumers (DVE evictions) drain
                        # fast, unlike the exp-gated sc ring.
                        tp = ps_pj.tile([128, HD], MMDT, tag="pj", name="tp")
                        nc.tensor.transpose(tp[:], kvt[0:64, bass.ts(jj, 128)],
                                            ident[:])
                        nc.vector.tensor_copy(vnat_sb[:, jt, 0:HD], tp[:])
                    return tpc
                for jj in range(NB // 128):
                    chunks.append(tpc_mk(jj))
                return chunks

            def make_oproj_chunks(it):
                last = it == IBLK - 1   # tail: nothing left to overlap, so
                chunks = []             # split evictions DVE/ACT + both queues
                for sti in range(it * (IB // 128), (it + 1) * (IB // 128)):
                    og = opool.tile([128, H], MMDT, tag="og", name=f"og{sti}")
                    for eb in range(H // 512):
                        def opc(sti=sti, eb=eb, og=og):
                            ssl = bass.ts(sti, 128)
                            op = ps_pj.tile([128, 512], F32, tag="pj", name="op")
                            nc.tensor.matmul(op[:], aoT_sb[:, 0, ssl],
                                             wo_sb[:, 0, bass.ts(eb, 512)],
                                             start=True, stop=False)
                            nc.tensor.matmul(op[:], aoT_sb[:, 1, ssl],
                                             wo_sb[:, 1, bass.ts(eb, 512)],
                                             start=False, stop=True)
                            if last and eb % 2 == 1:
                                nc.scalar.copy(og[:, bass.ts(eb, 512)], op[:])
                            else:
                                nc.vector.tensor_copy(og[:, bass.ts(eb, 512)], op[:])
                            if eb == H // 512 - 1:
                                if last and sti % 2 == 1:
                                    nc.scalar.dma_start(out_d[ssl, :], og[:])
                                else:
                                    nc.sync.dma_start(out_d[ssl, :], og[:])
                        chunks.append(opc)
                return chunks

            def attention(it, chunks):
                i_lo = it * IB
                njt = (it + 1) * (IB // 128)
                norm_chunks = []

                def emit_scores(mt, jt):
                    i0 = max(0, jt * 128 - i_lo)
                    st = ps_sc.tile([128, 2, IB], F32, tag="sc", name="st")
                    nc.tensor.matmul(
                        st[:, 0, i0:IB], kT_sb[0:64, bass.ts(jt, 128)],
                        qT_sb[0:64, mt, i_lo + i0:i_lo + IB],
                        start=True, stop=True)
                    nc.tensor.matmul(
                        st[:, 1, i0:IB], kT_sb[64:128, bass.ts(jt, 128)],
                        qT_sb[64:128, mt, i_lo + i0:i_lo + IB],
                        start=True, stop=True)
                    ex = expool.tile([128, 2, IB], MMDT, tag="ex", name="ex")
                    nc.scalar.activation(
                        ex[:, :, i0:IB], st[:, :, i0:IB],
                        mybir.ActivationFunctionType.Exp, scale=1.0 / 8.0)
                    if jt * 128 >= i_lo:  # mask the triangular boundary
                        nc.gpsimd.tensor_tensor(
                            ex[:, :, i0:i0 + 128], ex[:, :, i0:i0 + 128],
                            tri_sb[:], mybir.AluOpType.mult)
                    return ex, i0

                for mt in range(2):  # head pair on partitions 0:64 / 64:128
                    pv0 = ps_pv.tile([HD + 1, IB], F32, tag="pv", name="pv0")
                    pv1 = ps_pv.tile([HD + 1, IB], F32, tag="pv", name="pv1")
                    # software-pipelined by one j-tile: scores(jt+1) is emitted
                    # before pv(jt) so exp(jt) latency hides behind real PE work
                    pipe = emit_scores(mt, 0)
                    for jt in range(njt):
                        if jt + 1 < njt:
                            nxt = emit_scores(mt, jt + 1)
                        else:
                            nxt = None
                        if chunks:
                            chunks.pop(0)()  # PE filler while ACT runs exp
                        ex, i0 = pipe
                        nc.tensor.matmul(
                            pv0[:, i0:IB], vnat_sb[:, jt, :], ex[:, 0, i0:IB],
                            start=(jt == 0), stop=(jt == njt - 1))
                        nc.tensor.matmul(
                            pv1[:, i0:IB], vnat_sb[:, jt, :], ex[:, 1, i0:IB],
                            start=(jt == 0), stop=(jt == njt - 1))
                        pipe = nxt
                    # softmax denominators: ln from PSUM, reciprocal via exp(-x)
                    lz = aopool.tile([1, 2 * IB], F32, tag="lz", name="lz")
                    nc.scalar.activation(lz[0:1, 0:IB], pv0[HD:HD + 1, :],
                                         mybir.ActivationFunctionType.Ln)
                    nc.scalar.activation(lz[0:1, IB:2 * IB], pv1[HD:HD + 1, :],
                                         mybir.ActivationFunctionType.Ln)
                    ao = aopool.tile([128, IB], F32, tag="ao", name="ao")
                    nc.vector.tensor_copy(ao[0:HD, :], pv0[0:HD, :])
                    nc.scalar.copy(ao[64:64 + HD, :], pv1[0:HD, :])

                    # defer rr + the bc matmuls + normalize so they don't
                    # head-of-line-block the ACT/PE queues behind the chain
                    # (rr reads only SBUF lz, so it can safely pop later)
                    def norm(mt=mt, lz=lz, ao=ao):
                        rr = aopool.tile([1, 2 * IB], MMDT, tag="rr", name="rr")
                        nc.scalar.activation(rr[:], lz[:],
                                             mybir.ActivationFunctionType.Exp,
                                             scale=-1.0)
                        # sc-ring slots: st accumulations are single-emission,
                        # so a popped-in bc write can never split an open chain
                        bc0 = ps_sc.tile([128, 2, IB], F32, tag="sc", name="bc0")
                        nc.tensor.matmul(bc0[:, 0, :], ones128[:], rr[0:1, 0:IB],
                                         start=True, stop=True)
                        nc.vector.tensor_tensor(
                            aoT_sb[0:HD, mt, i_lo:i_lo + IB], ao[0:HD, :],
                            bc0[0:HD, 0, :], mybir.AluOpType.mult)
                        bc1 = ps_sc.tile([128, 2, IB], F32, tag="sc", name="bc1")
                        nc.tensor.matmul(bc1[:, 0, :], ones128[:],
                                         rr[0:1, IB:2 * IB],
                                         start=True, stop=True)
                        nc.vector.tensor_tensor(
                            aoT_sb[64:64 + HD, mt, i_lo:i_lo + IB],
                            ao[64:64 + HD, :], bc1[64:128, 0, :],
                            mybir.AluOpType.mult)
                    if mt == 0:
                        chunks.insert(min(2, len(chunks)), norm)
                    else:
                        norm_chunks.append(norm)
                return norm_chunks

            # ---- main pipeline ----
            # startup: sync queue streams wq/xt0 interleaved then wkv
            # (compute-critical path); scalar queue streams rope tables +
            # constants + wo in parallel.
            chunks0 = make_proj_chunks(0)      # issues wq+xt0 DMAs
            nc.sync.dma_start(wkv_sb[:], wkvd[:])
            nc.scalar.dma_start(cos_sb[:], cosd[:])
            nc.scalar.dma_start(sin_sb[:], sind[:])
            nc.scalar.dma_start(ident[:], identd[:])
            nc.scalar.dma_start(ones128[:], onesd[0:1, 0:128])
            nc.vector.memset(vnat_sb[:, :, HD:HD + 1], 1.0)
            nc.scalar.dma_start(tri_sb[:], trid[:])
            nc.scalar.dma_start(wo_sb[:], wod[:])
            for c in chunks0:
                c()
            opq = []                     # deferred o_proj chunks
            norm_pending = []
            for it in range(IBLK):
                chunks = list(norm_pending)
                take = (0, 8, 16, 24)[it]
                chunks += opq[:take]
                opq = opq[take:]
                if it + 1 < NBLK:
                    chunks += make_proj_chunks(it + 1)
                norm_pending = attention(it, chunks)
                for c in chunks:  # drain leftover proj work before next i-block
                    c()
                opq += make_oproj_chunks(it)
            for c in norm_pending:
                c()
            for c in opq:
                c()

    orig = nc.to_json_bytes
    nc.to_json_bytes = lambda: _split_multi_waits(orig())
    return nc


PERM64 = np.empty(64, dtype=np.int64)
PERM64[0::2] = np.arange(32)          # partition 2j   <- dim j
PERM64[1::2] = np.arange(32) + 32     # partition 2j+1 <- dim j+32


def _host_tables(position_ids):
    pos = np.asarray(position_ids).reshape(-1).astype(np.float64)
    inv = 1.0 / (ROPE_BASE ** (np.arange(0, HD, 2, dtype=np.float64) / HD))  # [32]
    fr = pos[None, :] * inv[:, None]                        # [32, S]
    c64 = np.empty((64, len(pos)))
    c64[0::2] = np.cos(fr)
    c64[1::2] = np.cos(fr)
    s64 = np.empty((64, len(pos)))
    s64[0::2] = -np.sin(fr)                                 # sign of rotate_half
    s64[1::2] = np.sin(fr)
    cosT = np.vstack([c64, c64]).astype(MMNP)               # [128, S]
    sinT = np.vstack([s64, s64]).astype(MMNP)
    tri = (np.arange(128)[:, None] <= np.arange(128)[None, :])
    tri01 = np.broadcast_to(tri[:, None, :], (128, 2, 128)).astype(MMNP)
    return cosT, sinT, np.ascontiguousarray(tri01)


def _perm_heads(w):
    """Permute each 64-col head block of w by PERM64 (pair dims adjacent)."""
    out = w.reshape(w.shape[0], -1, 64)[:, :, PERM64]
    return out.reshape(w.shape[0], -1)


_NC_CACHE = {}


def kernel(**inputs):
    x = np.asarray(inputs["x"], dtype=np.float32)
    wq = np.asarray(inputs["wq"], dtype=np.float32)
    wk = np.asarray(inputs["wk"], dtype=np.float32)
    wv = np.asarray(inputs["wv"], dtype=np.float32)
    wo = np.asarray(inputs["wo"], dtype=np.float32)
    cosT, sinT, tri01 = _host_tables(inputs["position_ids"])
    # x [S,H] -> [p, nb, ko, s'] so each per-partition DMA row is contiguous
    xTd = np.ascontiguousarray(
        x.reshape(NBLK, NB, KT, 128).transpose(3, 0, 2, 1)).astype(MMNP)

    if "nc" not in _NC_CACHE:
        _NC_CACHE["nc"] = build_nc()
    nc = _NC_CACHE["nc"]

    wqp = _perm_heads(wq)
    wkp = _perm_heads(wk)
    in_maps = []
    for c in range(NCORES):
        wq_c = wqp[:, c * QW:(c + 1) * QW]
        wkv_c = np.concatenate([wv[:, c * HD:(c + 1) * HD],
                                wkp[:, c * HD:(c + 1) * HD]], axis=1)  # [v | k]
        wo_c = wo[c * QW:(c + 1) * QW, :]
        in_maps.append({
            "xT": xTd,
            "wq": np.ascontiguousarray(
                wq_c.reshape(KT, 128, QW).transpose(1, 0, 2)).astype(MMNP),
            "wkv": np.ascontiguousarray(
                wkv_c.reshape(KT, 128, 128).transpose(1, 0, 2)).astype(MMNP),
            "wo": np.ascontiguousarray(
                wo_c.reshape(2, 128, H).transpose(1, 0, 2)).astype(MMNP),
            "cosT": cosT, "sinT": sinT, "tri01": tri01,
            "ident64": np.eye(64, dtype=MMNP),
            "ones": np.ones((128, 129), dtype=MMNP),
        })
    res = run_bass_kernel_spmd(nc, in_maps, core_ids=list(range(NCORES)))
    acc = np.zeros((S, H), dtype=np.float64)
    for c in range(NCORES):
        acc += res.results[c]["out"].astype(np.float64)
    return acc.astype(np.float32).reshape(B, S, H)


if __name__ == "__main__":
    rng = np.random.default_rng(0)
    ins = {
        "x": rng.standard_normal((B, S, H), dtype=np.float32),
        "position_ids": np.broadcast_to(np.arange(S, dtype=np.int64), (B, S)),
        "wq": (rng.standard_normal((H, NH * HD), dtype=np.float32) * 0.02),
        "wk": (rng.standard_normal((H, NKV * HD), dtype=np.float32) * 0.02),
        "wv": (rng.standard_normal((H, NKV * HD), dtype=np.float32) * 0.02),
        "wo": (rng.standard_normal((NH * HD, H), dtype=np.float32) * 0.02),
    }
    out = kernel(**ins)
    print(out.shape, out.dtype, np.abs(out).mean())
